# revision 23
# baseline (speedup 1.0000x reference)
"""Multi-head attention (B=2, T=2048, C=1024, H=16, D=64) on 8 TRN2 NeuronCores.

Sharding: tensor-parallel over heads x data-parallel over batch.
Core i = (b, g) with b = i // 4 (batch), g = i % 4 (head-group of 4 heads
= 256 channels). Each core computes, for its batch:
  Q.T/K.T = (Wg @ x.T) + b   (bf16 matmuls, f32 psum)
  S.T = Kh.T^T @ Qh.T        (scores transposed: [k, q] layout)
  P.T = exp(S.T / 8)         (no max-subtraction: scores are O(1) here)
  [out.T; denom] = [Vh | 1]^T @ P.T   (ones-column computes softmax denom)
  attn.T = out.T / denom
  partial = attn.T^T @ WoT_g (per-group slice of the output projection)
Host gathers: out[b] = sum_g partial[b, g] + bo.

Activations are cast to bf16 and laid out [C, T] on the host (TensorE
contracts over the partition dim, so x must arrive channel-major); all
weight slicing/transposition is host-side sharding prep as well.
"""

import numpy as np
import ml_dtypes

import concourse.bass as bass  # noqa: F401
import concourse.mybir as mybir
import concourse.tile as tile
from concourse import bacc
from concourse.bass_utils import run_bass_kernel_spmd

B, T, C = 2, 2048, 1024
H = 16
D = C // H  # 64
G = 4  # head-groups (cores per batch)
HPG = H // G  # heads per group = 4
CH = HPG * D  # channels per group = 256
KT = C // 128  # 8 contraction tiles
TB = T // 128  # 16 T-blocks
BF = mybir.dt.bfloat16
F32 = mybir.dt.float32

_CACHED_NC = None


def _build():
    nc = bacc.Bacc("TRN2", num_swdge_queues=4)
    xqt = nc.dram_tensor("xqt", [C, T], BF, kind="ExternalInput")
    xkt = nc.dram_tensor("xkt", [C, T], BF, kind="ExternalInput")
    xvt = nc.dram_tensor("xvt", [C, T], BF, kind="ExternalInput")
    wq = nc.dram_tensor("wq", [C, CH], BF, kind="ExternalInput")
    wk = nc.dram_tensor("wk", [C, CH], BF, kind="ExternalInput")
    wv = nc.dram_tensor("wv", [C, CH], BF, kind="ExternalInput")
    wo = nc.dram_tensor("wo", [CH, C], BF, kind="ExternalInput")
    bq = nc.dram_tensor("bq", [128, 2], F32, kind="ExternalInput")
    bk = nc.dram_tensor("bk", [128, 2], F32, kind="ExternalInput")
    bv = nc.dram_tensor("bv", [1, CH], F32, kind="ExternalInput")
    out = nc.dram_tensor("out", [T, C], F32, kind="ExternalOutput")

    with tile.TileContext(nc) as tc:
        with (
            tc.tile_pool(name="pw", bufs=1) as pw,
            tc.tile_pool(name="pxT", bufs=1) as pxT,
            tc.tile_pool(name="pqk", bufs=1) as pqk,
            tc.tile_pool(name="ppt", bufs=8) as ppt,
            tc.tile_pool(name="pnorm", bufs=2) as pnorm,
            tc.tile_pool(name="pobuf", bufs=4) as pobuf,
            tc.tile_pool(name="pdram", bufs=2, space="DRAM") as pdram,
            tc.tile_pool(name="psA", bufs=2, space="PSUM") as psA,
            tc.tile_pool(name="psB", bufs=2, space="PSUM") as psB,
        ):
            # ---- weights & biases to SBUF (SWDGE queues; HWDGE carries
            # the big x loads) ----
            wq_sb = pw.tile([128, KT, CH], BF)
            wk_sb = pw.tile([128, KT, CH], BF)
            wv_sb = pw.tile([128, KT, CH], BF)
            wo_sb = pw.tile([128, 2, C], BF)
            nc.gpsimd.dma_start(out=wv_sb, in_=wv.rearrange("(kt p) c -> p kt c", p=128))
            nc.gpsimd.dma_start(out=wq_sb, in_=wq.rearrange("(kt p) c -> p kt c", p=128))
            nc.gpsimd.dma_start(out=wk_sb, in_=wk.rearrange("(kt p) c -> p kt c", p=128))
            nc.gpsimd.dma_start(out=wo_sb, in_=wo.rearrange("(a p) c -> p a c", p=128))
            bq_sb = pw.tile([128, 2], F32)
            bk_sb = pw.tile([128, 2], F32)
            bv_sb = pw.tile([128, CH], F32)
            nc.gpsimd.dma_start(out=bq_sb, in_=bq[:, :])
            nc.gpsimd.dma_start(out=bk_sb, in_=bk[:, :])
            nc.gpsimd.dma_start(out=bv_sb, in_=bv[:, :].to_broadcast([128, CH]))

            # ---- x arrives host-transposed [C, T]; contiguous DMA loads,
            # xv first (V gates the attention), then xk, xq ----
            xqT = pxT.tile([128, KT, T], BF)
            xkT = pxT.tile([128, KT, T], BF)
            xvT = pxT.tile([128, KT, T], BF)
            for xT, src in ((xvT, xvt), (xkT, xkt), (xqT, xqt)):
                for kt in range(KT):
                    nc.sync.dma_start(
                        out=xT[:, kt, :], in_=src[kt * 128 : (kt + 1) * 128, :]
                    )

            # per-half tiles so attention h0/h1 doesn't falsely depend on
            # the a=1 projections (Tile deps are whole-tile)
            qTh = [pqk.tile([128, T], BF, tag=f"qT{a}", name=f"qT{a}") for a in range(2)]
            kTh = [pqk.tile([128, T], BF, tag=f"kT{a}", name=f"kT{a}") for a in range(2)]
            attnTh = [pqk.tile([128, T], BF, tag=f"at{a}", name=f"at{a}") for a in range(2)]
            # V natural layout [t within block, tb, head, d | ones]
            vnat = pqk.tile([128, TB, HPG, D + 1], BF)
            nc.vector.memset(vnat[:, :, :, D : D + 1], 1.0)

            def v_proj():
                for tb in range(TB):
                    ps = psA.tile([128, 1024], F32, tag="ps")
                    for kt in range(KT):
                        nc.tensor.matmul(
                            ps[:, 0:CH],
                            lhsT=xvT[:, kt, tb * 128 : (tb + 1) * 128],
                            rhs=wv_sb[:, kt, :],
                            start=(kt == 0),
                            stop=(kt == KT - 1),
                        )
                    nc.vector.tensor_add(
                        vnat[:, tb, :, 0:D],
                        ps[:, 0:CH].rearrange("p (h d) -> p h d", h=HPG),
                        bv_sb.rearrange("p (h d) -> p h d", h=HPG),
                    )

            def proj_one(dsts, w_sb, b_sb, xT, a):
                # one of K/Q for one half; DVE evac with per-partition bias
                if True:
                    for th in range(2):
                        ps = psA.tile([128, 1024], F32, tag="ps")
                        for kt in range(KT):
                            for cch in range(2):
                                nc.tensor.matmul(
                                    ps[:, cch * 512 : (cch + 1) * 512],
                                    lhsT=w_sb[:, kt, a * 128 : (a + 1) * 128],
                                    rhs=xT[:, kt, th * 1024 + cch * 512 : th * 1024 + (cch + 1) * 512],
                                    start=(kt == 0),
                                    stop=(kt == KT - 1),
                                )
                        nc.vector.tensor_scalar_add(
                            out=dsts[a][:, th * 1024 : (th + 1) * 1024],
                            in0=ps,
                            scalar1=b_sb[:, a : a + 1],
                        )

            def attention(h):
                prow = slice((h % 2) * 64, (h % 2) * 64 + 64)
                a = h // 2
                for qh in range(2):
                    qsl = slice(qh * 1024, (qh + 1) * 1024)
                    av = psB.tile([128, 1024], F32, tag="av")
                    for kb in range(TB):
                        s = psA.tile([128, 1024], F32, tag="ps")
                        for cq in range(2):
                            nc.tensor.matmul(
                                s[:, cq * 512 : (cq + 1) * 512],
                                lhsT=kTh[a][prow, kb * 128 : (kb + 1) * 128],
                                rhs=qTh[a][prow, qh * 1024 + cq * 512 : qh * 1024 + (cq + 1) * 512],
                                start=True,
                                stop=True,
                            )
                        pt = ppt.tile([128, 1024], BF)
                        nc.scalar.activation(
                            out=pt, in_=s,
                            func=mybir.ActivationFunctionType.Exp,
                            scale=0.125,
                        )
                        for cq in range(2):
                            nc.tensor.matmul(
                                av[0 : D + 1, cq * 512 : (cq + 1) * 512],
                                lhsT=vnat[:, kb, h, :],
                                rhs=pt[:, cq * 512 : (cq + 1) * 512],
                                start=(kb == 0),
                                stop=(kb == TB - 1),
                            )
                    # denom row -> DRAM -> broadcast to 64 partitions, then
                    # full-width reciprocal + multiply (a [1,N] DVE op would
                    # run on a single lane: 6.5us)
                    den = pnorm.tile([1, 1024], F32, tag="den")
                    nc.vector.tensor_copy(den, av[D : D + 1, :])
                    rcd = pdram.tile([1, 1024], F32)
                    nc.gpsimd.dma_start(out=rcd, in_=den)
                    rcb = pnorm.tile([64, 1024], F32)
                    nc.gpsimd.dma_start(out=rcb, in_=rcd.to_broadcast([64, 1024]))
                    rcr = pnorm.tile([64, 1024], F32)
                    nc.vector.reciprocal(rcr, rcb)
                    nc.vector.tensor_mul(attnTh[a][prow, qsl], av[0:D, :], rcr)

            v_proj()
            proj_one(kTh, wk_sb, bk_sb, xkT, 0)
            proj_one(qTh, wq_sb, bq_sb, xqT, 0)
            proj_one(kTh, wk_sb, bk_sb, xkT, 1)
            proj_one(qTh, wq_sb, bq_sb, xqT, 1)
            attention(0)
            attention(1)
            attention(2)
            attention(3)

            # ---- output projection (partial: this group's channels) ----
            for tb in range(TB):
                ps = psA.tile([128, 1024], F32, tag="ps")
                for a in range(2):
                    for cc in range(2):
                        nc.tensor.matmul(
                            ps[:, cc * 512 : (cc + 1) * 512],
                            lhsT=attnTh[a][:, tb * 128 : (tb + 1) * 128],
                            rhs=wo_sb[:, a, cc * 512 : (cc + 1) * 512],
                            start=(a == 0),
                            stop=(a == 1),
                        )
                ob = pobuf.tile([128, 1024], F32)
                nc.scalar.copy(ob[:, 0:512], ps[:, 0:512])
                nc.vector.tensor_copy(ob[:, 512:1024], ps[:, 512:1024])
                nc.sync.dma_start(out=out[tb * 128 : (tb + 1) * 128, :], in_=ob)

    nc.finalize()
    return nc


def kernel(query, key, value, Wq, bq, Wk, bk, Wv, bv, Wo, bo, _trace=False):
    global _CACHED_NC
    if _CACHED_NC is None:
        _CACHED_NC = _build()
    nc = _CACHED_NC

    bf16 = ml_dtypes.bfloat16
    x_bf = {
        "xqt": [np.ascontiguousarray(np.asarray(query[b], np.float32).T).astype(bf16) for b in range(B)],
        "xkt": [np.ascontiguousarray(np.asarray(key[b], np.float32).T).astype(bf16) for b in range(B)],
        "xvt": [np.ascontiguousarray(np.asarray(value[b], np.float32).T).astype(bf16) for b in range(B)],
    }
    Wq, Wk, Wv, Wo = (np.asarray(w, np.float32) for w in (Wq, Wk, Wv, Wo))
    bq, bk, bv, bo = (np.asarray(v, np.float32) for v in (bq, bk, bv, bo))
    wslices = []
    for g in range(G):
        cs = slice(g * CH, (g + 1) * CH)
        wslices.append(
            {
                "wq": np.ascontiguousarray(Wq[cs, :].T).astype(bf16),
                "wk": np.ascontiguousarray(Wk[cs, :].T).astype(bf16),
                "wv": np.ascontiguousarray(Wv[cs, :].T).astype(bf16),
                "wo": np.ascontiguousarray(Wo[:, cs].T).astype(bf16),
                "bq": np.ascontiguousarray(bq[cs].reshape(2, 128).T),
                "bk": np.ascontiguousarray(bk[cs].reshape(2, 128).T),
                "bv": np.ascontiguousarray(bv[cs].reshape(1, CH)),
            }
        )
    in_maps = []
    for core in range(8):
        b, g = core // G, core % G
        m = {k: v[b] for k, v in x_bf.items()}
        m.update(wslices[g])
        in_maps.append(m)

    res = run_bass_kernel_spmd(nc, in_maps, core_ids=list(range(8)), trace=_trace)
    outs = np.stack([r["out"] for r in res.results])  # [8, T, C]
    full = outs.reshape(B, G, T, C).sum(axis=1, dtype=np.float32) + bo.astype(
        np.float32
    )
    if _trace:
        return full.astype(np.float32), res
    return full.astype(np.float32)
